# revision 1
# baseline (speedup 1.0000x reference)
"""Compact bilinear pooling kernel for 8 Trainium2 NeuronCores.

Algorithm (host side folds everything into matmul weights):
  out[b,:,n] = circconv_1024(S1 @ x1[b,:,n], S2 @ x2[b,:,n])
Decomposed via x^1024-1 = (x^512-1)(x^512+1):
  cyclic-512 branch (rFFT512) + negacyclic-512 branch (odd DFT), both fused
  with the count-sketch matrices into dense real forward matrices
  W_j [512c -> 1024 freq rows], applied as f32r matmuls. Middle (complex
  multiply) runs on bf16 SBUF tiles on the vector engine. Inverse transforms
  are two block-diagonal [512 rows -> 512 outs] bf16 matmuls; the final
  unfold (c+d, c-d) happens on the vector engine reading inverse PSUM.

Sharding: batch 32 -> 4 per core (data parallel), weights replicated.
Layout: channels/freq rows on SBUF partitions, positions on free axis.
No transposes anywhere.
"""
import sys

sys.path.insert(0, "/opt/trn_rl_repo")

import numpy as np
import concourse.bass as bass
import concourse.mybir as mybir
from concourse import bacc
from concourse.tile import TileContext
from concourse.bass_utils import run_bass_kernel_spmd

B, C, HW, O = 32, 512, 784, 1024
NCORES = 8
BPC = B // NCORES  # 4 batches per core
PT = 392  # positions per tile (784 = 2*392; tiles never cross batch bounds)
NT = BPC * HW // PT  # 8 pos tiles per core
H = O // 2  # 512
F32, F32R, BF16 = mybir.dt.float32, mybir.dt.float32r, mybir.dt.bfloat16


def _build_host_matrices(sketch1, sketch2):
    """Fused fwd [512 c, 1024 freq-rows]; inverse IE/IF [256,256], ID [512,512].

    Level-2 folded row layout (see numpy_check2.py): e=rfft256, f=oddDFT256,
    d=oddDFT512; inverse weights carry the unfold 1/2 factors.
    """

    def build_fwd(sketch):
        sk = np.asarray(sketch, dtype=np.float64)
        Sp = sk[:H] + sk[H:]
        Sm = sk[:H] - sk[H:]
        Spp = Sp[:256] + Sp[256:]
        Spm = Sp[:256] - Sp[256:]
        n2 = np.arange(256)[None, :]
        k2 = np.arange(129)[:, None]
        Mc2 = np.exp(-2j * np.pi * k2 * n2 / 256) @ Spp
        k2f = np.arange(128)[:, None]
        Mo2 = np.exp(-2j * np.pi * n2 * (2 * k2f + 1) / 512) @ Spm
        n = np.arange(H)[None, :]
        ko = np.arange(256)[:, None]
        Mo = np.exp(-2j * np.pi * n * (2 * ko + 1) / O) @ Sm
        W = np.zeros((O, C))
        W[0:128] = Mc2[0:128].real
        W[128] = Mc2[128].real
        W[129:256] = Mc2[1:128].imag
        W[256:384] = Mo2.real
        W[384:512] = Mo2.imag
        W[512:768] = Mo.real
        W[768:1024] = Mo.imag
        return np.ascontiguousarray(W.T).astype(np.float32)  # [C, O]

    j2 = np.arange(256)[None, :]
    k = np.arange(128)[:, None]
    IE = np.zeros((256, 256))
    IE[0:128] = 2 * np.cos(2 * np.pi * k * j2 / 256) / 256
    IE[0] = 1.0 / 256
    IE[128] = np.cos(np.pi * j2) / 256
    ki = np.arange(1, 128)[:, None]
    IE[129:256] = -2 * np.sin(2 * np.pi * ki * j2 / 256) / 256
    IF = np.zeros((256, 256))
    IF[0:128] = 2 * np.cos(2 * np.pi * (2 * k + 1) * j2 / 512) / 256
    IF[128:256] = -2 * np.sin(2 * np.pi * (2 * k + 1) * j2 / 512) / 256
    j = np.arange(H)[None, :]
    ko = np.arange(256)[:, None]
    ID = np.zeros((H, H))
    ID[0:256] = 2 * np.cos(2 * np.pi * (2 * ko + 1) * j / O) / H
    ID[256:512] = -2 * np.sin(2 * np.pi * (2 * ko + 1) * j / O) / H
    return (
        build_fwd(sketch1),
        build_fwd(sketch2),
        (IE / 4).astype(np.float32),
        (IF / 4).astype(np.float32),
        (ID / 2).astype(np.float32),
    )


def _build_program(cfg=None):
    cfg = cfg or {}
    psf_bufs = cfg.get("psf_bufs", 2)
    xbufs = cfg.get("xbufs", 2)
    fbufs = cfg.get("fbufs", 2)
    obufs = cfg.get("obufs", 2)
    xload = cfg.get("xload", "castdma")  # castdma | stage_gpsimd | stage_dve
    wload = cfg.get("wload", "castdma")  # castdma | staged
    fwd_dt = BF16 if cfg.get("fwd_bf16", True) else F32R
    nt_override = cfg.get("nt", NT)
    nc = bacc.Bacc(None)
    x1e = nc.declare_dram_parameter("x1", [BPC, C, HW], F32, isOutput=False)
    x2e = nc.declare_dram_parameter("x2", [BPC, C, HW], F32, isOutput=False)
    w1e = nc.declare_dram_parameter("w1", [C, O], F32, isOutput=False)
    w2e = nc.declare_dram_parameter("w2", [C, O], F32, isOutput=False)
    iee = nc.declare_dram_parameter("ie", [256, 256], F32, isOutput=False)
    ife = nc.declare_dram_parameter("if", [256, 256], F32, isOutput=False)
    ide = nc.declare_dram_parameter("id", [H, H], F32, isOutput=False)
    oute = nc.declare_dram_parameter("out", [BPC, O, HW], F32, isOutput=True)

    with TileContext(nc) as tc:
        with (
            tc.tile_pool(name="wpool", bufs=1) as wpool,
            tc.tile_pool(name="xpool", bufs=xbufs) as xpool,
            tc.tile_pool(name="fpool", bufs=fbufs) as fpool,
            tc.tile_pool(name="opool", bufs=obufs) as opool,
            tc.tile_pool(name="psf", bufs=psf_bufs, space="PSUM") as psf,
            tc.tile_pool(name="pse", bufs=cfg.get("pse_bufs", 2), space="PSUM") as pse,
            tc.tile_pool(name="psq", bufs=cfg.get("psq_bufs", 2), space="PSUM") as psq,
            tc.tile_pool(name="psd", bufs=cfg.get("psd_bufs", 2), space="PSUM") as psd,
        ):
            def load_x_j(t, b, nsl):
                pw = nsl.stop - nsl.start
                xr = {}
                for j, xe in ((1, x1e), (2, x2e)):
                    for cc in range(4):
                        xt = xpool.tile(
                            [128, PT], fwd_dt, tag=f"x{j}_{cc}", name=f"x{j}_{cc}_{t}"
                        )
                        if xload == "castdma":
                            nc.gpsimd.dma_start(
                                out=xt[:, :pw], in_=xe[b, cc * 128 : (cc + 1) * 128, nsl]
                            )
                        else:
                            xs = xpool.tile(
                                [128, PT], F32, tag=f"xs{j}_{cc}", name=f"xs{j}_{cc}_{t}"
                            )
                            nc.sync.dma_start(
                                out=xs[:, :pw], in_=xe[b, cc * 128 : (cc + 1) * 128, nsl]
                            )
                            ceng = nc.gpsimd if xload == "stage_gpsimd" else nc.vector
                            ceng.tensor_copy(xt[:, :pw], xs[:, :pw])
                        xr[(j, cc)] = xt
                return xr

            # optionally lead the DMA queues with tile-0 x loads
            _xr_pre = (
                {0: load_x_j(0, 0, slice(0, PT))} if cfg.get("x_first", False) else {}
            )

            # ---- weights (cast to matmul dtypes) ----
            w1r, w2r, iet, ift, idt = [], [], [], [], []
            specs = [(w1r, w1e, O, fwd_dt, "w1r", 4), (w2r, w2e, O, fwd_dt, "w2r", 4),
                     (iet, iee, 256, BF16, "ie", 2), (ift, ife, 256, BF16, "if", 2),
                     (idt, ide, H, BF16, "id", 4)]
            for lst, ext, shp, dt_, nm, nch in specs:
                for cc in range(nch):
                    sl = slice(cc * 128, (cc + 1) * 128)
                    t = wpool.tile([128, shp], dt_, tag=f"{nm}{cc}", name=f"{nm}{cc}")
                    if wload == "castdma":
                        nc.gpsimd.dma_start(out=t[:], in_=ext[sl])
                    else:
                        st = wpool.tile(
                            [128, shp], F32, tag=f"{nm}s{cc}", name=f"{nm}s{cc}"
                        )
                        nc.sync.dma_start(out=st[:], in_=ext[sl])
                        inv_mat = nm in ("ie", "if", "id")
                        use_act = cfg.get("wcast_act", False) or (
                            inv_mat and cfg.get("invcast_act", False)
                        )
                        wceng = nc.scalar if use_act else nc.vector
                        if wceng is nc.scalar:
                            wceng.copy(out=t[:], in_=st[:])
                        else:
                            wceng.tensor_copy(t[:], st[:])
                    lst.append(t)

            # ---- main loop over position tiles ----
            # split the final tile in half to shorten the serial tail
            jobs = [(t, (t // 2), (t % 2) * PT, PT) for t in range(nt_override)]
            if cfg.get("tail_split", False) and nt_override == NT:
                lt, lb, ln0, _ = jobs.pop()
                jobs.append((lt, lb, ln0, PT // 2))
                jobs.append((lt + 1, lb, ln0 + PT // 2, PT // 2))
            for t, b, n0, pw in jobs:
                nsl = slice(n0, n0 + pw)
                xr = _xr_pre[t] if t in _xr_pre else load_x_j(t, b, nsl)

                # forward: fft_j[fc] [128 freq, PT] bf16
                fft = {}
                for j, wr in ((1, w1r), (2, w2r)):
                    for fc in range(8):
                        ps = psf.tile([128, PT], F32, tag="psf", name=f"psf{j}_{fc}_{t}")
                        for cc in range(4):
                            nc.tensor.matmul(
                                ps[:, :pw],
                                wr[cc][:, fc * 128 : (fc + 1) * 128],
                                xr[(j, cc)][:, :pw],
                                start=(cc == 0),
                                stop=(cc == 3),
                            )
                        ft = fpool.tile(
                            [128, PT], BF16, tag=f"fft{j}_{fc}", name=f"fft{j}_{fc}_{t}"
                        )
                        nc.scalar.copy(out=ft[:, :pw], in_=ps[:, :pw])
                        fft[(j, fc)] = ft

                # complex multiply on DVE (bf16): chunk pairs (re,im)
                prod = {}
                for pair_i, (re_c, im_c) in enumerate(((0, 1), (2, 3), (4, 6), (5, 7))):
                    eng = nc.vector
                    a1, b1 = fft[(1, re_c)], fft[(1, im_c)]
                    a2, b2 = fft[(2, re_c)], fft[(2, im_c)]
                    m1 = fpool.tile([128, PT], BF16, tag="m1", name=f"m1_{re_c}_{t}")
                    m2 = fpool.tile([128, PT], BF16, tag="m2", name=f"m2_{re_c}_{t}")
                    pr = fpool.tile(
                        [128, PT], BF16, tag=f"pr{re_c}", name=f"pr{re_c}_{t}"
                    )
                    pi = fpool.tile(
                        [128, PT], BF16, tag=f"pi{im_c}", name=f"pi{im_c}_{t}"
                    )
                    W_ = slice(0, pw)
                    eng.tensor_mul(m1[:, W_], a1[:, W_], a2[:, W_])
                    eng.tensor_mul(m2[:, W_], b1[:, W_], b2[:, W_])
                    eng.tensor_sub(pr[:, W_], m1[:, W_], m2[:, W_])
                    eng.tensor_mul(m1[:, W_], a1[:, W_], b2[:, W_])
                    eng.tensor_mul(m2[:, W_], b1[:, W_], a2[:, W_])
                    eng.tensor_add(pi[:, W_], m1[:, W_], m2[:, W_])
                    if re_c == 0:
                        # row 0 of the (0,1) pair: DC_e (re) and Nyquist-256
                        # (held in im slot row 0) are real-only products
                        feng = nc.gpsimd if cfg.get("fix_gp", False) else eng
                        feng.tensor_mul(pr[0:1, W_], a1[0:1, W_], a2[0:1, W_])
                        feng.tensor_mul(pi[0:1, W_], b1[0:1, W_], b2[0:1, W_])
                    prod[re_c] = pr
                    prod[im_c] = pi

                # inverse level2: e,f [256] then c = unfold2(e,f) in SBUF
                cch = []
                for oc2 in range(2):
                    osl2 = slice(oc2 * 128, (oc2 + 1) * 128)
                    pe_ = pse.tile([128, PT], F32, tag="pse", name=f"pse{oc2}_{t}")
                    pf_ = psq.tile([128, PT], F32, tag="psq", name=f"psq{oc2}_{t}")
                    for rc in range(2):
                        nc.tensor.matmul(
                            pe_[:, :pw], iet[rc][:, osl2], prod[rc][:, :pw],
                            start=(rc == 0), stop=(rc == 1),
                        )
                    for rc in range(2):
                        nc.tensor.matmul(
                            pf_[:, :pw], ift[rc][:, osl2], prod[2 + rc][:, :pw],
                            start=(rc == 0), stop=(rc == 1),
                        )
                    es = opool.tile([128, PT], F32, tag=f"es{oc2}", name=f"es{oc2}_{t}")
                    nc.scalar.copy(out=es[:, :pw], in_=pe_[:, :pw])
                    cch.append((es, pf_))
                cs = []
                for oc in range(4):
                    es, pf_ = cch[oc % 2]
                    ct = opool.tile([128, PT], F32, tag=f"c{oc}", name=f"c{oc}_{t}")
                    if oc < 2:
                        nc.vector.tensor_add(ct[:, :pw], es[:, :pw], pf_[:, :pw])
                    else:
                        nc.vector.tensor_sub(ct[:, :pw], es[:, :pw], pf_[:, :pw])
                    cs.append(ct)

                # inverse d + final unfold + store
                for oc in range(4):
                    osl = slice(oc * 128, (oc + 1) * 128)
                    pd = psd.tile([128, PT], F32, tag="psd", name=f"psd{oc}_{t}")
                    for rc in range(4):
                        nc.tensor.matmul(
                            pd[:, :pw], idt[rc][:, osl], prod[4 + rc][:, :pw],
                            start=(rc == 0), stop=(rc == 3),
                        )
                    lo = opool.tile([128, PT], F32, tag=f"lo{oc}", name=f"lo{oc}_{t}")
                    hi = opool.tile([128, PT], F32, tag=f"hi{oc}", name=f"hi{oc}_{t}")
                    nc.vector.tensor_add(lo[:, :pw], cs[oc][:, :pw], pd[:, :pw])
                    nc.vector.tensor_sub(hi[:, :pw], cs[oc][:, :pw], pd[:, :pw])
                    nc.sync.dma_start(out=oute[b, osl, nsl], in_=lo[:, :pw])
                    nc.sync.dma_start(
                        out=oute[b, slice(512 + oc * 128, 512 + (oc + 1) * 128), nsl],
                        in_=hi[:, :pw],
                    )

    nc.finalize()
    return nc


_NC_CACHE = None
KCFG = {"psf_bufs": 3, "pse_bufs": 1, "fbufs": 3, "wload": "staged"}


def kernel(x1, x2, sketch1, sketch2):
    global _NC_CACHE
    w1, w2, ie, if_, idm = _build_host_matrices(sketch1, sketch2)
    if _NC_CACHE is None:
        _NC_CACHE = _build_program(KCFG)
    nc = _NC_CACHE
    x1f = np.ascontiguousarray(np.asarray(x1, dtype=np.float32).reshape(B, C, HW))
    x2f = np.ascontiguousarray(np.asarray(x2, dtype=np.float32).reshape(B, C, HW))
    in_maps = []
    for i in range(NCORES):
        bs = slice(i * BPC, (i + 1) * BPC)
        in_maps.append(
            {
                "x1": np.ascontiguousarray(x1f[bs]),
                "x2": np.ascontiguousarray(x2f[bs]),
                "w1": w1,
                "w2": w2,
                "ie": ie,
                "if": if_,
                "id": idm,
            }
        )
    res = run_bass_kernel_spmd(nc, in_maps, list(range(NCORES)))
    out = np.concatenate([res.results[i]["out"] for i in range(NCORES)], axis=0)
    return out.reshape(B, O, 28, 28).astype(np.float32)


if __name__ == "__main__":
    rng = np.random.default_rng(0)
    x1 = rng.standard_normal((B, C, 28, 28)).astype(np.float32)
    x2 = rng.standard_normal((B, C, 28, 28)).astype(np.float32)
    h1 = rng.integers(0, O, C)
    s1 = rng.integers(0, 2, C) * 2.0 - 1.0
    h2 = rng.integers(0, O, C)
    s2 = rng.integers(0, 2, C) * 2.0 - 1.0
    sk1 = np.zeros((O, C), np.float32)
    sk1[h1, np.arange(C)] = s1
    sk2 = np.zeros((O, C), np.float32)
    sk2[h2, np.arange(C)] = s2
    got = kernel(x1, x2, sk1, sk2)
    p1 = np.einsum("bchw,oc->bohw", x1, sk1).reshape(B, O, HW)
    p2 = np.einsum("bchw,oc->bohw", x2, sk2).reshape(B, O, HW)
    ref = np.fft.ifft(np.fft.fft(p1, axis=1) * np.fft.fft(p2, axis=1), axis=1).real
    err = np.abs(got.reshape(B, O, HW) - ref).max() / np.abs(ref).max()
    print("self-test max rel err:", err)



# revision 20
# speedup vs baseline: 1.1637x; 1.1637x over previous
"""Compact bilinear pooling kernel for 8 Trainium2 NeuronCores.

Algorithm (host side folds everything into matmul weights):
  out[b,:,n] = circconv_1024(S1 @ x1[b,:,n], S2 @ x2[b,:,n])
Decomposed via x^1024-1 = (x^512-1)(x^512+1):
  cyclic-512 branch (rFFT512) + negacyclic-512 branch (odd DFT), both fused
  with the count-sketch matrices into dense real forward matrices
  W_j [512c -> 1024 freq rows].

v2: the forward matmuls run in fp8 DoubleRow perf mode (2 k-chunks per
pass, 0.5 cyc/row) using a 3-term hi/lo split W@x ~= Wh@xh + Wh@xl + Wl@xh
with Wh = fp8(W), Wl = fp8(W - Wh) (and likewise xh/xl, split on the host).
Residual error ~0.03% -- better than a plain bf16 matmul. This takes the
forward from 16 bf16 passes to 12 DR passes per (input, freq-chunk) at half
cost each: 25% fewer PE cycles overall vs bf16.

Middle (complex multiply) runs on bf16 SBUF tiles on the vector engine.
Inverse transforms are block-diagonal bf16 matmuls [IE/IF 256x256, ID
512x512]; unfolds run on DVE/GPSIMD; output is stored bf16 and widened on
the host.

Sharding: batch 32 -> 4 per core (data parallel), weights replicated.
Layout: channels/freq rows on SBUF partitions, positions on free axis.
"""
import sys

sys.path.insert(0, "/opt/trn_rl_repo")

import ml_dtypes
import numpy as np
import concourse.bass as bass
import concourse.mybir as mybir
from concourse import bacc
from concourse.tile import TileContext
from concourse.bass_utils import run_bass_kernel_spmd

B, C, HW, O = 32, 512, 784, 1024
NCORES = 8
BPC = B // NCORES  # 4 batches per core
PT = 392  # positions per tile (784 = 2*392; tiles never cross batch bounds)
NT = BPC * HW // PT  # 8 pos tiles per core
H = O // 2  # 512
F32, BF16, E4 = mybir.dt.float32, mybir.dt.bfloat16, mybir.dt.float8e4
DR = mybir.MatmulPerfMode.DoubleRow
E4NP = ml_dtypes.float8_e4m3
BF16NP = ml_dtypes.bfloat16


def _build_host_matrices(sketch1, sketch2):
    """Fused fwd [512 c, 1024 freq-rows]; inverse IE/IF [256,256], ID [512,512].

    Level-2 folded row layout: e=rfft256, f=oddDFT256, d=oddDFT512; inverse
    weights carry the unfold 1/2 factors.
    """

    def build_fwd(sketch):
        sk = np.asarray(sketch, dtype=np.float64)
        Sp = sk[:H] + sk[H:]
        Sm = sk[:H] - sk[H:]
        Spp = Sp[:256] + Sp[256:]
        Spm = Sp[:256] - Sp[256:]
        n2 = np.arange(256)[None, :]
        k2 = np.arange(129)[:, None]
        Mc2 = np.exp(-2j * np.pi * k2 * n2 / 256) @ Spp
        k2f = np.arange(128)[:, None]
        Mo2 = np.exp(-2j * np.pi * n2 * (2 * k2f + 1) / 512) @ Spm
        n = np.arange(H)[None, :]
        ko = np.arange(256)[:, None]
        Mo = np.exp(-2j * np.pi * n * (2 * ko + 1) / O) @ Sm
        W = np.zeros((O, C))
        W[0:128] = Mc2[0:128].real
        W[128] = Mc2[128].real
        W[129:256] = Mc2[1:128].imag
        W[256:384] = Mo2.real
        W[384:512] = Mo2.imag
        W[512:768] = Mo.real
        W[768:1024] = Mo.imag
        return np.ascontiguousarray(W.T).astype(np.float32)  # [C, O]

    j2 = np.arange(256)[None, :]
    k = np.arange(128)[:, None]
    IE = np.zeros((256, 256))
    IE[0:128] = 2 * np.cos(2 * np.pi * k * j2 / 256) / 256
    IE[0] = 1.0 / 256
    IE[128] = np.cos(np.pi * j2) / 256
    ki = np.arange(1, 128)[:, None]
    IE[129:256] = -2 * np.sin(2 * np.pi * ki * j2 / 256) / 256
    IF = np.zeros((256, 256))
    IF[0:128] = 2 * np.cos(2 * np.pi * (2 * k + 1) * j2 / 512) / 256
    IF[128:256] = -2 * np.sin(2 * np.pi * (2 * k + 1) * j2 / 512) / 256
    j = np.arange(H)[None, :]
    ko = np.arange(256)[:, None]
    ID = np.zeros((H, H))
    ID[0:256] = 2 * np.cos(2 * np.pi * (2 * ko + 1) * j / O) / H
    ID[256:512] = -2 * np.sin(2 * np.pi * (2 * ko + 1) * j / O) / H
    return (
        build_fwd(sketch1),
        build_fwd(sketch2),
        (IE / 4).astype(np.float32),
        (IF / 4).astype(np.float32),
        (ID / 2).astype(np.float32),
    )


def _pack_fwd_w(w):
    """[C, O] f32 -> (hi, lo) packed [128 p, 2 kp, 2 i, 8 fc, 128 m] fp8.

    c = (2*kp + i)*128 + p, o = fc*128 + m. The (i, fc, m) block per (p, kp)
    is contiguous so DMA runs are 2048 bytes."""
    wh32 = w.astype(E4NP).astype(np.float32)
    wl = (w - wh32).astype(E4NP)
    wh = w.astype(E4NP)

    def pack(a):
        a5 = a.reshape(2, 2, 128, 8, 128)  # [kp, i, p, fc, m]
        return a5.transpose(3, 2, 0, 1, 4)  # [fc, p, kp, i, m]

    return np.ascontiguousarray(np.stack([pack(wh), pack(wl)], axis=1))


def _split_x(x):
    """[N, C, HW] f32 -> [N, 2, C, HW] fp8 (hi, lo)."""
    xh = x.astype(E4NP)
    xl = (x - xh.astype(np.float32)).astype(E4NP)
    return np.ascontiguousarray(np.stack([xh, xl], axis=1))


def _build_program(cfg=None):
    cfg = cfg or {}
    psf_bufs = cfg.get("psf_bufs", 4)
    pse_bufs = cfg.get("pse_bufs", 1)
    psq_bufs = cfg.get("psq_bufs", 1)
    psd_bufs = cfg.get("psd_bufs", 2)
    xbufs = cfg.get("xbufs", 2)
    fbufs = cfg.get("fbufs", 2)
    obufs = cfg.get("obufs", 2)
    pool_ocs = tuple(cfg.get("pool_ocs", (0, 1)))  # final-unfold ocs on gpsimd
    # per-batch position tile widths; first/last shaped to shorten the
    # pipeline fill/drain
    warm = cfg.get("warm", (196, 392, 196))
    taper = cfg.get("taper", (392, 196, 196))
    last_dve = cfg.get("last_dve", True)

    nc = bacc.Bacc(None)
    x1e = nc.declare_dram_parameter("x1", [BPC, 2, C, HW], E4, isOutput=False)
    x2e = nc.declare_dram_parameter("x2", [BPC, 2, C, HW], E4, isOutput=False)
    wde = {}
    for nm in ("w1", "w2"):
        wde[nm] = nc.declare_dram_parameter(
            nm, [8, 2, 128, 2, 2, 128], E4, isOutput=False
        )
    iee = nc.declare_dram_parameter("ie", [256, 256], BF16, isOutput=False)
    ife = nc.declare_dram_parameter("if", [256, 256], BF16, isOutput=False)
    ide = nc.declare_dram_parameter("id", [H, H], BF16, isOutput=False)
    oute = nc.declare_dram_parameter("out", [BPC, O, HW], BF16, isOutput=True)

    with TileContext(nc) as tc:
        with (
            tc.tile_pool(name="wpool", bufs=1) as wpool,
            tc.tile_pool(name="xpool", bufs=xbufs) as xpool,
            tc.tile_pool(name="fpool", bufs=fbufs) as fpool,
            tc.tile_pool(name="opool", bufs=obufs) as opool,
            tc.tile_pool(name="psf", bufs=psf_bufs, space="PSUM") as psf,
            tc.tile_pool(name="pse", bufs=pse_bufs, space="PSUM") as pse,
            tc.tile_pool(name="psq", bufs=psq_bufs, space="PSUM") as psq,
            tc.tile_pool(name="psd", bufs=psd_bufs, space="PSUM") as psd,
        ):
            def load_x_j(t, b, nsl, queue=None, only_j=None):
                pw = nsl.stop - nsl.start
                xr = {}
                for j, xe in ((1, x1e), (2, x2e)):
                    if only_j is not None and j != only_j:
                        continue
                    xt = xpool.tile(
                        [128, 2, 4, PT], E4, tag=f"x{j}", name=f"x{j}_{t}"
                    )
                    src = xe[b, :, :, nsl].rearrange("hl (cc p) n -> p hl cc n", p=128)
                    (queue or nc.gpsimd).dma_start(out=xt[:, :, :, :pw], in_=src)
                    xr[j] = xt
                return xr

            # ---- startup: interleave tile-0 x with per-half weight loads so
            # the first forward group can start as early as possible ----
            jobs = []
            for b in range(BPC):
                widths = warm if b == 0 else (taper if b == BPC - 1 else (PT, PT))
                n0 = 0
                for w_ in widths:
                    jobs.append((len(jobs), b, n0, w_))
                    n0 += w_
                assert n0 == HW

            fw = {}
            for nm in ("w1", "w2"):
                # dram [fc, hl, p, kp, i, m]; SBUF tile [p, fc, hl, kp, i, m]
                fw[nm] = wpool.tile([128, 8, 2, 2, 2, 128], E4, tag=nm, name=nm)

            def load_w_half(nm, half, queue):
                fsl = slice(half * 4, half * 4 + 4)
                src = wde[nm][fsl].rearrange("fc hl p kp i m -> p fc hl kp i m")
                queue.dma_start(out=fw[nm][:, fsl], in_=src)

            _xr_pre = {}
            t0w = jobs[0][3]
            _xr_pre[0] = load_x_j(0, 0, slice(0, t0w), queue=nc.sync, only_j=1)
            load_w_half("w1", 0, nc.sync)
            _xr_pre[0].update(load_x_j(0, 0, slice(0, t0w), queue=nc.gpsimd, only_j=2))
            load_w_half("w1", 1, nc.sync)
            load_w_half("w2", 0, nc.sync)
            load_w_half("w2", 1, nc.sync)
            iet, ift, idt = [], [], []
            for lst, ext, shp, nm, nch in (
                (iet, iee, 256, "ie", 2),
                (ift, ife, 256, "if", 2),
                (idt, ide, H, "id", 4),
            ):
                for cc in range(nch):
                    sl = slice(cc * 128, (cc + 1) * 128)
                    tl = wpool.tile([128, shp], BF16, tag=f"{nm}{cc}", name=f"{nm}{cc}")
                    nc.sync.dma_start(out=tl[:, :], in_=ext[sl])
                    lst.append(tl)

            # ---- main loop over position tiles ----
            for t, b, n0, pw in jobs:
                nsl = slice(n0, n0 + pw)
                xr = _xr_pre[t] if t in _xr_pre else load_x_j(t, b, nsl)
                is_last = t == len(jobs) - 1

                # forward: fft_j[fc] [128 freq, PT] bf16 via fp8 DR 3-term
                fft = {}
                for j in (1, 2):
                    wt = fw[f"w{j}"]
                    xt = xr[j]
                    for fc in range(8):
                        ps = psf.tile([128, PT], F32, tag="psf", name=f"psf{j}_{fc}_{t}")
                        mm = []
                        for kp in range(2):
                            lh = wt[:, fc, 0, kp, :, :]
                            ll = wt[:, fc, 1, kp, :, :]
                            rh = xt[:, 0, 2 * kp : 2 * kp + 2, :pw]
                            rl = xt[:, 1, 2 * kp : 2 * kp + 2, :pw]
                            mm += [(lh, rh), (ll, rh), (lh, rl)]
                        for i, (l_, r_) in enumerate(mm):
                            nc.tensor.matmul(
                                ps[:, :pw], l_, r_,
                                start=(i == 0), stop=(i == len(mm) - 1),
                                perf_mode=DR,
                            )
                        ft = fpool.tile(
                            [128, PT], BF16, tag=f"fft{j}_{fc}", name=f"fft{j}_{fc}_{t}"
                        )
                        nc.scalar.copy(out=ft[:, :pw], in_=ps[:, :pw])
                        fft[(j, fc)] = ft

                # complex multiply on DVE (bf16): chunk pairs (re,im)
                prod = {}
                for re_c, im_c in ((0, 1), (2, 3), (4, 6), (5, 7)):
                    eng = nc.vector
                    a1, b1 = fft[(1, re_c)], fft[(1, im_c)]
                    a2, b2 = fft[(2, re_c)], fft[(2, im_c)]
                    m1 = fpool.tile([128, PT], BF16, tag="m1", name=f"m1_{re_c}_{t}")
                    m2 = fpool.tile([128, PT], BF16, tag="m2", name=f"m2_{re_c}_{t}")
                    pr = fpool.tile([128, PT], BF16, tag=f"pr{re_c}", name=f"pr{re_c}_{t}")
                    pi = fpool.tile([128, PT], BF16, tag=f"pi{im_c}", name=f"pi{im_c}_{t}")
                    W_ = slice(0, pw)
                    eng.tensor_mul(m1[:, W_], a1[:, W_], a2[:, W_])
                    eng.tensor_mul(m2[:, W_], b1[:, W_], b2[:, W_])
                    eng.tensor_sub(pr[:, W_], m1[:, W_], m2[:, W_])
                    eng.tensor_mul(m1[:, W_], a1[:, W_], b2[:, W_])
                    eng.tensor_mul(m2[:, W_], b1[:, W_], a2[:, W_])
                    eng.tensor_add(pi[:, W_], m1[:, W_], m2[:, W_])
                    if re_c == 0:
                        # row 0 of the (0,1) pair: DC_e (re) and Nyquist-256
                        # (held in im slot row 0) are real-only products
                        feng = nc.gpsimd if cfg.get("row0_pool", True) else eng
                        feng.tensor_mul(pr[0:1, W_], a1[0:1, W_], a2[0:1, W_])
                        feng.tensor_mul(pi[0:1, W_], b1[0:1, W_], b2[0:1, W_])
                    prod[re_c] = pr
                    prod[im_c] = pi

                # inverse level2: e,f [256] then c = unfold2(e,f) in SBUF
                cch = []
                for oc2 in range(2):
                    osl2 = slice(oc2 * 128, (oc2 + 1) * 128)
                    pe_ = pse.tile([128, PT], F32, tag="pse", name=f"pse{oc2}_{t}")
                    pf_ = psq.tile([128, PT], F32, tag="psq", name=f"psq{oc2}_{t}")
                    for rc in range(2):
                        nc.tensor.matmul(
                            pe_[:, :pw], iet[rc][:, osl2], prod[rc][:, :pw],
                            start=(rc == 0), stop=(rc == 1),
                        )
                    for rc in range(2):
                        nc.tensor.matmul(
                            pf_[:, :pw], ift[rc][:, osl2], prod[2 + rc][:, :pw],
                            start=(rc == 0), stop=(rc == 1),
                        )
                    es = opool.tile([128, PT], F32, tag=f"es{oc2}", name=f"es{oc2}_{t}")
                    nc.scalar.copy(out=es[:, :pw], in_=pe_[:, :pw])
                    cch.append((es, pf_))
                cs = []
                for oc in range(4):
                    es, pf_ = cch[oc % 2]
                    ct = opool.tile([128, PT], F32, tag=f"c{oc}", name=f"c{oc}_{t}")
                    if oc < 2:
                        nc.vector.tensor_add(ct[:, :pw], es[:, :pw], pf_[:, :pw])
                    else:
                        nc.vector.tensor_sub(ct[:, :pw], es[:, :pw], pf_[:, :pw])
                    cs.append(ct)

                # inverse d + final unfold + per-oc output DMA
                dst4 = oute[b].rearrange(
                    "(blk q p) n -> p blk q n", blk=2, q=4, p=128
                )
                oc_dma_q = {}
                if is_last and cfg.get("last_dma_split", True):
                    # drain: DVE-side ocs finish while the SP queue is still
                    # issuing the pool-side ocs -- give them their own queue
                    oc_dma_q = {oc: nc.scalar for oc in range(4) if oc not in pool_ocs}
                for oc in range(4):
                    osl = slice(oc * 128, (oc + 1) * 128)
                    pd = psd.tile([128, PT], F32, tag="psd", name=f"psd{oc}_{t}")
                    for rc in range(4):
                        nc.tensor.matmul(
                            pd[:, :pw], idt[rc][:, osl], prod[4 + rc][:, :pw],
                            start=(rc == 0), stop=(rc == 3),
                        )
                    ot = opool.tile([128, 2, PT], BF16, tag=f"ot{oc}", name=f"ot{oc}_{t}")
                    lo = ot[:, 0, :pw]
                    hi = ot[:, 1, :pw]
                    if oc in pool_ocs and not (is_last and last_dve):
                        # gpsimd has no PSUM port: stage d into SBUF first
                        dc = opool.tile([128, PT], F32, tag=f"dc{oc}", name=f"dc{oc}_{t}")
                        nc.scalar.copy(out=dc[:, :pw], in_=pd[:, :pw])
                        nc.gpsimd.tensor_add(lo, cs[oc][:, :pw], dc[:, :pw])
                        nc.gpsimd.tensor_sub(hi, cs[oc][:, :pw], dc[:, :pw])
                    else:
                        nc.vector.tensor_add(lo, cs[oc][:, :pw], pd[:, :pw])
                        nc.vector.tensor_sub(hi, cs[oc][:, :pw], pd[:, :pw])
                    nc.sync.dma_start(out=dst4[:, :, oc, nsl], in_=ot[:, :, :pw])

    nc.finalize()
    return nc


_NC_CACHE = None
KCFG = {
    "warm": (196, 196, 392),
    "taper": (392, 392),
    "last_dve": False,
    "row0_pool": False,
    "fbufs": 3,
}


def prepare_inputs(x1, x2, sketch1, sketch2):
    """Host-side packing shared by kernel() and the profiling harness."""
    w1, w2, ie, if_, idm = _build_host_matrices(sketch1, sketch2)
    w1p = _pack_fwd_w(w1)
    w2p = _pack_fwd_w(w2)
    ieb = ie.astype(BF16NP)
    ifb = if_.astype(BF16NP)
    idb = idm.astype(BF16NP)
    x1f = np.asarray(x1, dtype=np.float32).reshape(B, C, HW)
    x2f = np.asarray(x2, dtype=np.float32).reshape(B, C, HW)
    x1p = _split_x(x1f)
    x2p = _split_x(x2f)
    in_maps = []
    for i in range(NCORES):
        bs = slice(i * BPC, (i + 1) * BPC)
        in_maps.append(
            {
                "x1": np.ascontiguousarray(x1p[bs]),
                "x2": np.ascontiguousarray(x2p[bs]),
                "w1": w1p,
                "w2": w2p,
                "ie": ieb,
                "if": ifb,
                "id": idb,
            }
        )
    return in_maps


def kernel(x1, x2, sketch1, sketch2):
    global _NC_CACHE
    in_maps = prepare_inputs(x1, x2, sketch1, sketch2)
    if _NC_CACHE is None:
        _NC_CACHE = _build_program(KCFG)
    nc = _NC_CACHE
    res = run_bass_kernel_spmd(nc, in_maps, list(range(NCORES)))
    out = np.concatenate(
        [res.results[i]["out"].astype(np.float32) for i in range(NCORES)], axis=0
    )
    return out.reshape(B, O, 28, 28)


if __name__ == "__main__":
    rng = np.random.default_rng(0)
    x1 = rng.standard_normal((B, C, 28, 28)).astype(np.float32)
    x2 = rng.standard_normal((B, C, 28, 28)).astype(np.float32)
    h1 = rng.integers(0, O, C)
    s1 = rng.integers(0, 2, C) * 2.0 - 1.0
    h2 = rng.integers(0, O, C)
    s2 = rng.integers(0, 2, C) * 2.0 - 1.0
    sk1 = np.zeros((O, C), np.float32)
    sk1[h1, np.arange(C)] = s1
    sk2 = np.zeros((O, C), np.float32)
    sk2[h2, np.arange(C)] = s2
    got = kernel(x1, x2, sk1, sk2)
    p1 = np.einsum("bchw,oc->bohw", x1, sk1).reshape(B, O, HW)
    p2 = np.einsum("bchw,oc->bohw", x2, sk2).reshape(B, O, HW)
    ref = np.fft.ifft(np.fft.fft(p1, axis=1) * np.fft.fft(p2, axis=1), axis=1).real
    err = np.abs(got.reshape(B, O, HW) - ref).max() / np.abs(ref).max()
    print("self-test max rel err:", err)


# revision 24
# speedup vs baseline: 1.1694x; 1.0049x over previous
"""Compact bilinear pooling kernel for 8 Trainium2 NeuronCores.

Algorithm (host side folds everything into matmul weights):
  out[b,:,n] = circconv_1024(S1 @ x1[b,:,n], S2 @ x2[b,:,n])
Decomposed via x^1024-1 = (x^512-1)(x^512+1):
  cyclic-512 branch (rFFT512) + negacyclic-512 branch (odd DFT), both fused
  with the count-sketch matrices into dense real forward matrices
  W_j [512c -> 1024 freq rows].

v2: the forward matmuls run in fp8 DoubleRow perf mode (2 k-chunks per
pass, 0.5 cyc/row) using a 3-term hi/lo split W@x ~= Wh@xh + Wh@xl + Wl@xh
with Wh = fp8(W), Wl = fp8(W - Wh) (and likewise xh/xl, split on the host).
Residual error ~0.03% -- better than a plain bf16 matmul. This takes the
forward from 16 bf16 passes to 12 DR passes per (input, freq-chunk) at half
cost each: 25% fewer PE cycles overall vs bf16.

Middle (complex multiply) runs on bf16 SBUF tiles on the vector engine.
Inverse transforms are block-diagonal bf16 matmuls [IE/IF 256x256, ID
512x512]; unfolds run on DVE/GPSIMD; output is stored bf16 and widened on
the host.

Sharding: batch 32 -> 4 per core (data parallel), weights replicated.
Layout: channels/freq rows on SBUF partitions, positions on free axis.
"""
import sys

sys.path.insert(0, "/opt/trn_rl_repo")

import ml_dtypes
import numpy as np
import concourse.bass as bass
import concourse.mybir as mybir
from concourse import bacc
from concourse.tile import TileContext
from concourse.bass_utils import run_bass_kernel_spmd

B, C, HW, O = 32, 512, 784, 1024
NCORES = 8
BPC = B // NCORES  # 4 batches per core
PT = 392  # positions per tile (784 = 2*392; tiles never cross batch bounds)
NT = BPC * HW // PT  # 8 pos tiles per core
H = O // 2  # 512
F32, BF16, E4 = mybir.dt.float32, mybir.dt.bfloat16, mybir.dt.float8e4
DR = mybir.MatmulPerfMode.DoubleRow
E4NP = ml_dtypes.float8_e4m3
BF16NP = ml_dtypes.bfloat16


def _build_host_matrices(sketch1, sketch2):
    """Fused fwd [512 c, 1024 freq-rows]; inverse IE/IF [256,256], ID [512,512].

    Level-2 folded row layout: e=rfft256, f=oddDFT256, d=oddDFT512; inverse
    weights carry the unfold 1/2 factors.
    """

    def build_fwd(sketch):
        sk = np.asarray(sketch, dtype=np.float64)
        Sp = sk[:H] + sk[H:]
        Sm = sk[:H] - sk[H:]
        Spp = Sp[:256] + Sp[256:]
        Spm = Sp[:256] - Sp[256:]
        n2 = np.arange(256)[None, :]
        k2 = np.arange(129)[:, None]
        Mc2 = np.exp(-2j * np.pi * k2 * n2 / 256) @ Spp
        k2f = np.arange(128)[:, None]
        Mo2 = np.exp(-2j * np.pi * n2 * (2 * k2f + 1) / 512) @ Spm
        n = np.arange(H)[None, :]
        ko = np.arange(256)[:, None]
        Mo = np.exp(-2j * np.pi * n * (2 * ko + 1) / O) @ Sm
        W = np.zeros((O, C))
        W[0:128] = Mc2[0:128].real
        W[128] = Mc2[128].real
        W[129:256] = Mc2[1:128].imag
        W[256:384] = Mo2.real
        W[384:512] = Mo2.imag
        W[512:768] = Mo.real
        W[768:1024] = Mo.imag
        return np.ascontiguousarray(W.T).astype(np.float32)  # [C, O]

    j2 = np.arange(256)[None, :]
    k = np.arange(128)[:, None]
    IE = np.zeros((256, 256))
    IE[0:128] = 2 * np.cos(2 * np.pi * k * j2 / 256) / 256
    IE[0] = 1.0 / 256
    IE[128] = np.cos(np.pi * j2) / 256
    ki = np.arange(1, 128)[:, None]
    IE[129:256] = -2 * np.sin(2 * np.pi * ki * j2 / 256) / 256
    IF = np.zeros((256, 256))
    IF[0:128] = 2 * np.cos(2 * np.pi * (2 * k + 1) * j2 / 512) / 256
    IF[128:256] = -2 * np.sin(2 * np.pi * (2 * k + 1) * j2 / 512) / 256
    j = np.arange(H)[None, :]
    ko = np.arange(256)[:, None]
    ID = np.zeros((H, H))
    ID[0:256] = 2 * np.cos(2 * np.pi * (2 * ko + 1) * j / O) / H
    ID[256:512] = -2 * np.sin(2 * np.pi * (2 * ko + 1) * j / O) / H
    return (
        build_fwd(sketch1),
        build_fwd(sketch2),
        (IE / 4).astype(np.float32),
        (IF / 4).astype(np.float32),
        (ID / 2).astype(np.float32),
    )


def _pack_fwd_w(w):
    """[C, O] f32 -> (hi, lo) packed [128 p, 2 kp, 2 i, 8 fc, 128 m] fp8.

    c = (2*kp + i)*128 + p, o = fc*128 + m. The (i, fc, m) block per (p, kp)
    is contiguous so DMA runs are 2048 bytes."""
    wh32 = w.astype(E4NP).astype(np.float32)
    wl = (w - wh32).astype(E4NP)
    wh = w.astype(E4NP)

    def pack(a):
        a5 = a.reshape(2, 2, 128, 8, 128)  # [kp, i, p, fc, m]
        return a5.transpose(3, 2, 0, 1, 4)  # [fc, p, kp, i, m]

    return np.ascontiguousarray(np.stack([pack(wh), pack(wl)], axis=1))


def _split_x(x):
    """[N, C, HW] f32 -> [N, 2, C, HW] fp8 (hi, lo)."""
    xh = x.astype(E4NP)
    xl = (x - xh.astype(np.float32)).astype(E4NP)
    return np.ascontiguousarray(np.stack([xh, xl], axis=1))


def _build_program(cfg=None):
    cfg = cfg or {}
    psf_bufs = cfg.get("psf_bufs", 4)
    pse_bufs = cfg.get("pse_bufs", 1)
    psq_bufs = cfg.get("psq_bufs", 1)
    psd_bufs = cfg.get("psd_bufs", 2)
    xbufs = cfg.get("xbufs", 2)
    fbufs = cfg.get("fbufs", 2)
    obufs = cfg.get("obufs", 2)
    pool_ocs = tuple(cfg.get("pool_ocs", (0, 1)))  # final-unfold ocs on gpsimd
    # per-batch position tile widths; first/last shaped to shorten the
    # pipeline fill/drain
    warm = cfg.get("warm", (196, 392, 196))
    taper = cfg.get("taper", (392, 196, 196))
    last_dve = cfg.get("last_dve", True)

    nc = bacc.Bacc(None)
    x1e = nc.declare_dram_parameter("x1", [BPC, 2, C, HW], E4, isOutput=False)
    x2e = nc.declare_dram_parameter("x2", [BPC, 2, C, HW], E4, isOutput=False)
    wde = {}
    for nm in ("w1", "w2"):
        wde[nm] = nc.declare_dram_parameter(
            nm, [8, 2, 128, 2, 2, 128], E4, isOutput=False
        )
    iee = nc.declare_dram_parameter("ie", [256, 256], BF16, isOutput=False)
    ife = nc.declare_dram_parameter("if", [256, 256], BF16, isOutput=False)
    ide = nc.declare_dram_parameter("id", [H, H], BF16, isOutput=False)
    oute = nc.declare_dram_parameter("out", [BPC, O, HW], BF16, isOutput=True)

    with TileContext(nc) as tc:
        with (
            tc.tile_pool(name="wpool", bufs=1) as wpool,
            tc.tile_pool(name="xpool", bufs=xbufs) as xpool,
            tc.tile_pool(name="fpool", bufs=fbufs) as fpool,
            tc.tile_pool(name="opool", bufs=obufs) as opool,
            tc.tile_pool(name="psf", bufs=psf_bufs, space="PSUM") as psf,
            tc.tile_pool(name="pse", bufs=pse_bufs, space="PSUM") as pse,
            tc.tile_pool(name="psq", bufs=psq_bufs, space="PSUM") as psq,
            tc.tile_pool(name="psd", bufs=psd_bufs, space="PSUM") as psd,
        ):
            def load_x_j(t, b, nsl, queue=None, only_j=None):
                pw = nsl.stop - nsl.start
                xr = {}
                for j, xe in ((1, x1e), (2, x2e)):
                    if only_j is not None and j != only_j:
                        continue
                    xt = xpool.tile(
                        [128, 2, 4, PT], E4, tag=f"x{j}", name=f"x{j}_{t}"
                    )
                    src = xe[b, :, :, nsl].rearrange("hl (cc p) n -> p hl cc n", p=128)
                    (queue or nc.gpsimd).dma_start(out=xt[:, :, :, :pw], in_=src)
                    xr[j] = xt
                return xr

            # ---- startup: interleave tile-0 x with per-half weight loads so
            # the first forward group can start as early as possible ----
            jobs = []
            for b in range(BPC):
                widths = warm if b == 0 else (taper if b == BPC - 1 else (PT, PT))
                n0 = 0
                for w_ in widths:
                    jobs.append((len(jobs), b, n0, w_))
                    n0 += w_
                assert n0 == HW

            fw = {}
            for nm in ("w1", "w2"):
                # dram [fc, hl, p, kp, i, m]; SBUF tile [p, fc, hl, kp, i, m]
                fw[nm] = wpool.tile([128, 8, 2, 2, 2, 128], E4, tag=nm, name=nm)

            def load_w_half(nm, half, queue):
                fsl = slice(half * 4, half * 4 + 4)
                src = wde[nm][fsl].rearrange("fc hl p kp i m -> p fc hl kp i m")
                queue.dma_start(out=fw[nm][:, fsl], in_=src)

            _xr_pre = {}
            t0w = jobs[0][3]
            _xr_pre[0] = load_x_j(0, 0, slice(0, t0w), queue=nc.sync, only_j=1)
            load_w_half("w1", 0, nc.sync)
            _xr_pre[0].update(load_x_j(0, 0, slice(0, t0w), queue=nc.gpsimd, only_j=2))
            load_w_half("w1", 1, nc.sync)
            load_w_half("w2", 0, nc.sync)
            load_w_half("w2", 1, nc.sync)
            iet, ift, idt = [], [], []
            for lst, ext, shp, nm, nch in (
                (iet, iee, 256, "ie", 2),
                (ift, ife, 256, "if", 2),
                (idt, ide, H, "id", 4),
            ):
                for cc in range(nch):
                    sl = slice(cc * 128, (cc + 1) * 128)
                    tl = wpool.tile([128, shp], BF16, tag=f"{nm}{cc}", name=f"{nm}{cc}")
                    nc.sync.dma_start(out=tl[:, :], in_=ext[sl])
                    lst.append(tl)

            # ---- main loop over position tiles ----
            for t, b, n0, pw in jobs:
                nsl = slice(n0, n0 + pw)
                xr = _xr_pre[t] if t in _xr_pre else load_x_j(t, b, nsl)
                is_last = t == len(jobs) - 1

                # forward: fft_j[fc] [128 freq, PT] bf16 via fp8 DR 3-term
                fft = {}
                for j in (1, 2):
                    wt = fw[f"w{j}"]
                    xt = xr[j]
                    for fc in range(8):
                        ps = psf.tile([128, PT], F32, tag="psf", name=f"psf{j}_{fc}_{t}")
                        mm = []
                        for kp in range(2):
                            lh = wt[:, fc, 0, kp, :, :]
                            ll = wt[:, fc, 1, kp, :, :]
                            rh = xt[:, 0, 2 * kp : 2 * kp + 2, :pw]
                            rl = xt[:, 1, 2 * kp : 2 * kp + 2, :pw]
                            mm += [(lh, rh), (ll, rh), (lh, rl)]
                        for i, (l_, r_) in enumerate(mm):
                            nc.tensor.matmul(
                                ps[:, :pw], l_, r_,
                                start=(i == 0), stop=(i == len(mm) - 1),
                                perf_mode=DR,
                            )
                        ft = fpool.tile(
                            [128, PT], BF16, tag=f"fft{j}_{fc}", name=f"fft{j}_{fc}_{t}"
                        )
                        nc.scalar.copy(out=ft[:, :pw], in_=ps[:, :pw])
                        fft[(j, fc)] = ft

                # complex multiply on DVE (bf16): chunk pairs (re,im)
                prod = {}
                for re_c, im_c in ((0, 1), (2, 3), (4, 6), (5, 7)):
                    eng = nc.vector
                    a1, b1 = fft[(1, re_c)], fft[(1, im_c)]
                    a2, b2 = fft[(2, re_c)], fft[(2, im_c)]
                    m1 = fpool.tile([128, PT], BF16, tag="m1", name=f"m1_{re_c}_{t}")
                    m2 = fpool.tile([128, PT], BF16, tag="m2", name=f"m2_{re_c}_{t}")
                    pr = fpool.tile([128, PT], BF16, tag=f"pr{re_c}", name=f"pr{re_c}_{t}")
                    pi = fpool.tile([128, PT], BF16, tag=f"pi{im_c}", name=f"pi{im_c}_{t}")
                    W_ = slice(0, pw)
                    eng.tensor_mul(m1[:, W_], a1[:, W_], a2[:, W_])
                    eng.tensor_mul(m2[:, W_], b1[:, W_], b2[:, W_])
                    eng.tensor_sub(pr[:, W_], m1[:, W_], m2[:, W_])
                    eng.tensor_mul(m1[:, W_], a1[:, W_], b2[:, W_])
                    eng.tensor_mul(m2[:, W_], b1[:, W_], a2[:, W_])
                    eng.tensor_add(pi[:, W_], m1[:, W_], m2[:, W_])
                    if re_c == 0:
                        # row 0 of the (0,1) pair: DC_e (re) and Nyquist-256
                        # (held in im slot row 0) are real-only products
                        feng = nc.gpsimd if cfg.get("row0_pool", True) else eng
                        feng.tensor_mul(pr[0:1, W_], a1[0:1, W_], a2[0:1, W_])
                        feng.tensor_mul(pi[0:1, W_], b1[0:1, W_], b2[0:1, W_])
                    prod[re_c] = pr
                    prod[im_c] = pi

                # inverse level2: e,f [256] then c = unfold2(e,f) in SBUF
                cch = []
                for oc2 in range(2):
                    osl2 = slice(oc2 * 128, (oc2 + 1) * 128)
                    pe_ = pse.tile([128, PT], F32, tag="pse", name=f"pse{oc2}_{t}")
                    pf_ = psq.tile([128, PT], F32, tag="psq", name=f"psq{oc2}_{t}")
                    for rc in range(2):
                        nc.tensor.matmul(
                            pe_[:, :pw], iet[rc][:, osl2], prod[rc][:, :pw],
                            start=(rc == 0), stop=(rc == 1),
                        )
                    for rc in range(2):
                        nc.tensor.matmul(
                            pf_[:, :pw], ift[rc][:, osl2], prod[2 + rc][:, :pw],
                            start=(rc == 0), stop=(rc == 1),
                        )
                    es = opool.tile([128, PT], F32, tag=f"es{oc2}", name=f"es{oc2}_{t}")
                    nc.scalar.copy(out=es[:, :pw], in_=pe_[:, :pw])
                    cch.append((es, pf_))
                cs = []
                for oc in range(4):
                    es, pf_ = cch[oc % 2]
                    ct = opool.tile([128, PT], F32, tag=f"c{oc}", name=f"c{oc}_{t}")
                    if oc < 2:
                        nc.vector.tensor_add(ct[:, :pw], es[:, :pw], pf_[:, :pw])
                    else:
                        nc.vector.tensor_sub(ct[:, :pw], es[:, :pw], pf_[:, :pw])
                    cs.append(ct)

                # inverse d + final unfold + per-oc output DMA
                dst4 = oute[b].rearrange(
                    "(blk q p) n -> p blk q n", blk=2, q=4, p=128
                )
                oc_dma_q = {}
                oc_order = range(4)
                if t >= len(jobs) - cfg.get("dma_split_jobs", 1) and cfg.get(
                    "last_dma_split", True
                ):
                    # drain: DVE-side ocs finish while the SP queue is still
                    # issuing the pool-side ocs -- give them their own queue
                    # and run them first
                    oc_dma_q = {oc: nc.scalar for oc in range(4) if oc not in pool_ocs}
                    if cfg.get("last_oc_reorder", False):
                        oc_order = sorted(range(4), key=lambda oc: oc in pool_ocs)
                for oc in oc_order:
                    osl = slice(oc * 128, (oc + 1) * 128)
                    pd = psd.tile([128, PT], F32, tag="psd", name=f"psd{oc}_{t}")
                    for rc in range(4):
                        nc.tensor.matmul(
                            pd[:, :pw], idt[rc][:, osl], prod[4 + rc][:, :pw],
                            start=(rc == 0), stop=(rc == 3),
                        )
                    ot = opool.tile([128, 2, PT], BF16, tag=f"ot{oc}", name=f"ot{oc}_{t}")
                    lo = ot[:, 0, :pw]
                    hi = ot[:, 1, :pw]
                    if oc in pool_ocs and not (is_last and last_dve):
                        # gpsimd has no PSUM port: stage d into SBUF first
                        dc = opool.tile([128, PT], F32, tag=f"dc{oc}", name=f"dc{oc}_{t}")
                        nc.scalar.copy(out=dc[:, :pw], in_=pd[:, :pw])
                        nc.gpsimd.tensor_add(lo, cs[oc][:, :pw], dc[:, :pw])
                        nc.gpsimd.tensor_sub(hi, cs[oc][:, :pw], dc[:, :pw])
                    else:
                        nc.vector.tensor_add(lo, cs[oc][:, :pw], pd[:, :pw])
                        nc.vector.tensor_sub(hi, cs[oc][:, :pw], pd[:, :pw])
                    q = oc_dma_q.get(oc, nc.sync)
                    q.dma_start(out=dst4[:, :, oc, nsl], in_=ot[:, :, :pw])

    nc.finalize()
    return nc


_NC_CACHE = None
KCFG = {
    "warm": (196, 196, 392),
    "taper": (392, 392),
    "last_dve": False,
    "row0_pool": False,
    "fbufs": 3,
}


def prepare_inputs(x1, x2, sketch1, sketch2):
    """Host-side packing shared by kernel() and the profiling harness."""
    w1, w2, ie, if_, idm = _build_host_matrices(sketch1, sketch2)
    w1p = _pack_fwd_w(w1)
    w2p = _pack_fwd_w(w2)
    ieb = ie.astype(BF16NP)
    ifb = if_.astype(BF16NP)
    idb = idm.astype(BF16NP)
    x1f = np.asarray(x1, dtype=np.float32).reshape(B, C, HW)
    x2f = np.asarray(x2, dtype=np.float32).reshape(B, C, HW)
    x1p = _split_x(x1f)
    x2p = _split_x(x2f)
    in_maps = []
    for i in range(NCORES):
        bs = slice(i * BPC, (i + 1) * BPC)
        in_maps.append(
            {
                "x1": np.ascontiguousarray(x1p[bs]),
                "x2": np.ascontiguousarray(x2p[bs]),
                "w1": w1p,
                "w2": w2p,
                "ie": ieb,
                "if": ifb,
                "id": idb,
            }
        )
    return in_maps


def kernel(x1, x2, sketch1, sketch2):
    global _NC_CACHE
    in_maps = prepare_inputs(x1, x2, sketch1, sketch2)
    if _NC_CACHE is None:
        _NC_CACHE = _build_program(KCFG)
    nc = _NC_CACHE
    res = run_bass_kernel_spmd(nc, in_maps, list(range(NCORES)))
    out = np.concatenate(
        [res.results[i]["out"].astype(np.float32) for i in range(NCORES)], axis=0
    )
    return out.reshape(B, O, 28, 28)


if __name__ == "__main__":
    rng = np.random.default_rng(0)
    x1 = rng.standard_normal((B, C, 28, 28)).astype(np.float32)
    x2 = rng.standard_normal((B, C, 28, 28)).astype(np.float32)
    h1 = rng.integers(0, O, C)
    s1 = rng.integers(0, 2, C) * 2.0 - 1.0
    h2 = rng.integers(0, O, C)
    s2 = rng.integers(0, 2, C) * 2.0 - 1.0
    sk1 = np.zeros((O, C), np.float32)
    sk1[h1, np.arange(C)] = s1
    sk2 = np.zeros((O, C), np.float32)
    sk2[h2, np.arange(C)] = s2
    got = kernel(x1, x2, sk1, sk2)
    p1 = np.einsum("bchw,oc->bohw", x1, sk1).reshape(B, O, HW)
    p2 = np.einsum("bchw,oc->bohw", x2, sk2).reshape(B, O, HW)
    ref = np.fft.ifft(np.fft.fft(p1, axis=1) * np.fft.fft(p2, axis=1), axis=1).real
    err = np.abs(got.reshape(B, O, HW) - ref).max() / np.abs(ref).max()
    print("self-test max rel err:", err)
